# revision 18
# baseline (speedup 1.0000x reference)
"""Trainium2 Bass kernel for nn_Decoder (single-step attention LSTM decoder).

B=128, S=128, H=1024, E=512, V=32000, 8 NeuronCores.

Sharding:
  - attention: data-parallel over batch (16 rows per core)
  - LSTM: sharded over hidden units (128 h-rows per core), AllGather of context
    before and of h_new after
  - fc/logits + embedding table: vocab-parallel (4000 rows per core)

All matmuls run in fp16 (full PE rate, ~2^-11 rounding); accumulation is fp32
in PSUM. The attention score path (tanh -> v-dot -> softmax) is kept in fp32.

Host side only reshapes/transposes/casts/shards; all model math (embedding
gather included) runs on device.
"""
import numpy as np

import concourse.bass as bass
import concourse.mybir as mybir
import concourse.tile as tile
from concourse import bacc
from concourse.bass_utils import run_bass_kernel_spmd
from concourse.masks import make_identity

B, S, H, E, V = 128, 128, 1024, 512, 32000
NC = 8
BL = B // NC          # 16 local batch rows
HBL = H // NC         # 128 hidden rows per core (LSTM shard)
VS = V // NC          # 4000 vocab rows per core
KT_H = H // 128       # 8 k-tiles over H
KT_E = E // 128       # 4 k-tiles over E
KT_X = (E + 2 * H) // 128   # 20 k-tiles over [emb, context, h_last]
KT_2H = (2 * H) // 128      # 16 k-tiles over [h_new, context]
FC_CH = [512] * 7 + [VS - 512 * 7]   # psum-bank chunks of the 4000-wide shard

_CACHE = {}


def _build():
    nc = bacc.Bacc("TRN2", target_bir_lowering=False, debug=False,
                   num_devices=NC)
    dt = mybir.dt
    f32, f16, i32 = dt.float32, dt.float16, dt.int32
    AF = mybir.ActivationFunctionType
    OP = mybir.AluOpType

    # ---- I/O ----
    x_idx = nc.dram_tensor("x_idx", [B, 1], i32, kind="ExternalInput")
    emb_tab = nc.dram_tensor("emb_tab", [V, E], f32, kind="ExternalInput")
    hT_loc = nc.dram_tensor("hT_loc", [H, BL], f16, kind="ExternalInput")
    hT_full = nc.dram_tensor("hT_full", [H, B], f16, kind="ExternalInput")
    cT_blk = nc.dram_tensor("cT_blk", [HBL, B], f32, kind="ExternalInput")
    enc_nat = nc.dram_tensor("enc_nat", [BL * S, H], f16, kind="ExternalInput")
    encT = nc.dram_tensor("encT", [BL * H, S], f16, kind="ExternalInput")
    w1t = nc.dram_tensor("w1t", [H, H], f16, kind="ExternalInput")
    w2t = nc.dram_tensor("w2t", [H, H], f16, kind="ExternalInput")
    attnb = nc.dram_tensor("attnb", [1, H], f16, kind="ExternalInput")
    v_row = nc.dram_tensor("v_row", [1, H], f16, kind="ExternalInput")
    wcombT = nc.dram_tensor("wcombT", [E + 2 * H, 4 * HBL], f16,
                            kind="ExternalInput")
    bias_g = nc.dram_tensor("bias_g", [HBL, 4], f32, kind="ExternalInput")
    fcwT = nc.dram_tensor("fcwT", [2 * H, VS], f16, kind="ExternalInput")
    fcb = nc.dram_tensor("fcb", [1, VS], f16, kind="ExternalInput")

    logits_sh = nc.dram_tensor("logits_sh", [B, VS], f32, kind="ExternalOutput")
    attnw_sh = nc.dram_tensor("attnw_sh", [BL, S], f32, kind="ExternalOutput")
    h_sh = nc.dram_tensor("h_sh", [B, HBL], f32, kind="ExternalOutput")
    c_sh = nc.dram_tensor("c_sh", [B, HBL], f32, kind="ExternalOutput")

    with tile.TileContext(nc) as tc:
        with (
            tc.tile_pool(name="setup", bufs=1) as setup,
            tc.tile_pool(name="fcw", bufs=1) as fcwp,
            tc.tile_pool(name="w2p", bufs=1) as w2p,

            tc.tile_pool(name="encp", bufs=3) as encp,
            tc.tile_pool(name="natp", bufs=3) as natp,
            tc.tile_pool(name="energyp", bufs=2) as energyp,
            tc.tile_pool(name="p1bp", bufs=2) as p1bp,
            tc.tile_pool(name="wcp", bufs=4) as wcp,
            tc.tile_pool(name="lgp", bufs=2) as lgp,
            tc.tile_pool(name="smallp", bufs=1) as smallp,
            tc.tile_pool(name="lstmp", bufs=8) as lstmp,
            tc.tile_pool(name="psp", bufs=8, space="PSUM") as psp,
            tc.tile_pool(name="dram", bufs=1, space="DRAM") as dram,
        ):
            # ======== setup ========
            # fc weights: allocate + DMA first so the loads overlap attention
            fcw_sb = []
            for kt in range(KT_2H):
                t = fcwp.tile([128, VS], f16, name=f"fcw{kt}")
                nc.sync.dma_start(t[:], fcwT.ap()[kt * 128:(kt + 1) * 128, :])
                fcw_sb.append(t)

            ident32 = setup.tile([128, 128], f32)
            make_identity(nc, ident32[:])
            ones16 = setup.tile([1, 128], f16)
            nc.gpsimd.memset(ones16[:], 1.0)

            earlyp_cm = tc.tile_pool(name="earlyp", bufs=1)
            earlyp = earlyp_cm.__enter__()
            v_sb = earlyp.tile([1, H], f16)
            nc.sync.dma_start(v_sb[:], v_row.ap()[:, :])
            attnb_sb = earlyp.tile([1, H], f16)
            nc.sync.dma_start(attnb_sb[:], attnb.ap()[:, :])
            hTl_sb = setup.tile([128, KT_H, BL], f16)
            nc.sync.dma_start(
                hTl_sb[:], hT_loc.ap().rearrange("(kt p) b -> p kt b", p=128))
            hTf_sb = setup.tile([128, KT_H, B], f16)
            nc.sync.dma_start(
                hTf_sb[:], hT_full.ap().rearrange("(kt p) b -> p kt b", p=128))
            cT_sb = setup.tile([HBL, B], f32)
            nc.sync.dma_start(cT_sb[:], cT_blk.ap()[:, :])
            biasg_sb = setup.tile([HBL, 4], f32)
            nc.sync.dma_start(biasg_sb[:], bias_g.ap()[:, :])

            # embedding gather + transpose to [E, B] fp16 k-tiles
            idx_sb = setup.tile([B, 1], i32)
            nc.sync.dma_start(idx_sb[:], x_idx.ap()[:, :])
            embT_sb = setup.tile([128, KT_E, B], f16)
            with tc.tile_pool(name="embp", bufs=1) as embp:
                emb_bt = embp.tile([B, E], f32)
                nc.gpsimd.indirect_dma_start(
                    out=emb_bt[:], out_offset=None, in_=emb_tab.ap()[:, :],
                    in_offset=bass.IndirectOffsetOnAxis(ap=idx_sb[:, :1],
                                                        axis=0))
                for et in range(KT_E):
                    p = psp.tile([128, B], f32, tag="ps", name="embT_ps")
                    nc.tensor.transpose(p[:], emb_bt[:, bass.ts(et, 128)],
                                        ident32[:])
                    nc.scalar.copy(embT_sb[:, et, :], p[:])

            # W2^T resident: [128, kt, H]
            w2_sb = w2p.tile([128, KT_H, H], f16)
            for kt in range(KT_H):
                nc.sync.dma_start(w2_sb[:, kt, :],
                                  w2t.ap()[kt * 128:(kt + 1) * 128, :])

            # v broadcast to all partitions: ones^T @ v_row
            vb_sb = setup.tile([128, H], f16)
            for ch in range(2):
                vb_ps = psp.tile([128, 512], f32, tag="ps", name="vb_ps")
                nc.tensor.matmul(vb_ps[:], ones16[:, :128],
                                 v_sb[:, bass.ts(ch, 512)], start=True, stop=True)
                nc.vector.tensor_copy(vb_sb[:, bass.ts(ch, 512)], vb_ps[:])

            # ======== part1b: (h_last @ W1^T + attn_b) for local rows ========
            p1_rows = setup.tile([BL, H], f16)
            p1_chs = [psp.tile([BL, 512], f32, tag="ps", name=f"p1_ps{ch}")
                      for ch in range(2)]
            with tc.tile_pool(name="w1p", bufs=2) as w1p:
                for kt in range(KT_H):
                    w1_t = w1p.tile([128, H], f16, name="w1t_t")
                    nc.sync.dma_start(w1_t[:],
                                      w1t.ap()[kt * 128:(kt + 1) * 128, :])
                    for ch in range(2):
                        nc.tensor.matmul(p1_chs[ch][:],
                                         hTl_sb[:, kt, :],
                                         w1_t[:, bass.ts(ch, 512)],
                                         start=(kt == 0), stop=False)
            for ch in range(2):
                nc.tensor.matmul(p1_chs[ch][:], ones16[:, :BL],
                                 attnb_sb[:, bass.ts(ch, 512)],
                                 start=False, stop=True)
                nc.scalar.copy(p1_rows[:, bass.ts(ch, 512)], p1_chs[ch][:])
            earlyp_cm.__exit__(None, None, None)

            # ======== attention: energy / score per local row ========
            score_sb = setup.tile([128, BL], f32)
            for b in range(BL):
                encT_b = encp.tile([128, KT_H, S], f16, name="encT_b")
                nc.sync.dma_start(
                    encT_b[:],
                    encT.ap()[b * H:(b + 1) * H, :]
                    .rearrange("(kt p) s -> p kt s", p=128))
                # this row's part1 bias, moved onto partition 0
                p1b = p1bp.tile([1, H], f16, name="p1b")
                nc.sync.dma_start(p1b[:], p1_rows[b:b + 1, :])

                energy = energyp.tile([S, H], f16, name="energy")
                for ch in range(2):
                    pre_ps = psp.tile([S, 512], f32, tag="ps", name="pre_ps")
                    for kt in range(KT_H):
                        nc.tensor.matmul(
                            pre_ps[:], encT_b[:, kt, :],
                            w2_sb[:, kt, bass.ds(ch * 512, 512)],
                            start=(kt == 0), stop=False)
                    nc.tensor.matmul(
                        pre_ps[:], ones16[:, :S],
                        p1b[:, bass.ds(ch * 512, 512)],
                        start=False, stop=True)
                    nc.scalar.activation(energy[:, bass.ts(ch, 512)],
                                         pre_ps[:], AF.Tanh)
                nc.vector.tensor_mul(energy[:], energy[:], vb_sb[:])
                nc.vector.reduce_sum(out=score_sb[:, b:b + 1], in_=energy[:],
                                     axis=mybir.AxisListType.X)

            # ======== softmax over S (rows are local b) ========
            sc_ps = psp.tile([BL, S], f32, tag="ps", name="sc_ps")
            nc.tensor.transpose(sc_ps[:], score_sb[:, :BL], ident32[:])
            sc16 = smallp.tile([BL, S], f32)
            nc.scalar.copy(sc16[:], sc_ps[:])
            mx = smallp.tile([BL, 1], f32)
            nc.vector.reduce_max(out=mx[:], in_=sc16[:],
                                 axis=mybir.AxisListType.X)
            mxn = smallp.tile([BL, 1], f32)
            nc.scalar.mul(mxn[:], mx[:], -1.0)
            esc = smallp.tile([BL, S], f32)
            nc.scalar.activation(esc[:], sc16[:], AF.Exp, bias=mxn[:, :1])
            sm = smallp.tile([BL, 1], f32)
            nc.vector.reduce_sum(out=sm[:], in_=esc[:],
                                 axis=mybir.AxisListType.X)
            rs = smallp.tile([BL, 1], f32)
            nc.vector.reciprocal(rs[:], sm[:])
            w16 = smallp.tile([BL, S], f32)
            nc.vector.tensor_scalar_mul(w16[:], esc[:], rs[:, :1])
            nc.sync.dma_start(attnw_sh.ap()[:, :], w16[:])

            # ======== context = attn_weights @ enc ========
            wt_ps = psp.tile([S, BL], f32, tag="ps", name="wt_ps")
            nc.tensor.transpose(wt_ps[:], w16[:], ident32[:BL, :BL])
            wT = smallp.tile([S, BL], f16)
            nc.scalar.copy(wT[:], wt_ps[:])
            cc1_in = dram.tile([BL, H], f32)
            for b in range(BL):
                enc_b = natp.tile([S, H], f16, name="enc_b")
                nc.sync.dma_start(enc_b[:], enc_nat.ap()[b * S:(b + 1) * S, :])
                for ch in range(2):
                    ctx_ps = psp.tile([1, 512], f32, tag="ps", name="ctx_ps")
                    nc.tensor.matmul(ctx_ps[:],
                                     wT[:, b:b + 1],
                                     enc_b[:, bass.ts(ch, 512)],
                                     start=True, stop=True)
                    ctmp = p1bp.tile([1, 512], f32, tag="ctmp", name="ctmp")
                    nc.scalar.copy(ctmp[:], ctx_ps[:])
                    nc.sync.dma_start(
                        cc1_in[b:b + 1, bass.ds(ch * 512, 512)], ctmp[:])

            # ======== AllGather context -> full [B, H] ========
            cc1_out = dram.tile([B, H], f32, addr_space="Shared")
            nc.gpsimd.collective_compute(
                "AllGather", mybir.AluOpType.bypass,
                replica_groups=[list(range(NC))],
                ins=[cc1_in[:].opt()], outs=[cc1_out[:].opt()])
            ctx_full = setup.tile([B, H], f32)
            nc.sync.dma_start(ctx_full[:], cc1_out[:])
            ctxT_sb = setup.tile([128, KT_H, B], f16)
            for ht in range(KT_H):
                p = psp.tile([128, B], f32, tag="ps", name="ctxT_ps")
                nc.tensor.transpose(p[:], ctx_full[:, bass.ts(ht, 128)],
                                    ident32[:])
                nc.scalar.copy(ctxT_sb[:, ht, :], p[:])

            # ======== LSTM (gates sharded over h-block) ========
            rhs_tiles = ([embT_sb[:, et, :] for et in range(KT_E)]
                         + [ctxT_sb[:, ht, :] for ht in range(KT_H)]
                         + [hTf_sb[:, ht, :] for ht in range(KT_H)])
            g_ps = [psp.tile([HBL, B], f32, tag="ps", name=f"g_ps{g}")
                    for g in range(4)]
            for kt in range(KT_X):
                wc_t = wcp.tile([128, 4 * HBL], f16, name="wc_t")
                nc.sync.dma_start(wc_t[:],
                                  wcombT.ap()[kt * 128:(kt + 1) * 128, :])
                for g in range(4):
                    nc.tensor.matmul(g_ps[g][:], wc_t[:, bass.ts(g, HBL)],
                                     rhs_tiles[kt],
                                     start=(kt == 0), stop=(kt == KT_X - 1))
            i_s = lstmp.tile([HBL, B], f32, tag="lstm", name="i_s")
            f_s = lstmp.tile([HBL, B], f32, tag="lstm", name="f_s")
            gg = lstmp.tile([HBL, B], f32, tag="lstm", name="gg")
            o_s = lstmp.tile([HBL, B], f32, tag="lstm", name="o_s")
            nc.scalar.activation(i_s[:], g_ps[0][:], AF.Sigmoid,
                                 bias=biasg_sb[:, 0:1])
            nc.scalar.activation(f_s[:], g_ps[1][:], AF.Sigmoid,
                                 bias=biasg_sb[:, 1:2])
            nc.scalar.activation(gg[:], g_ps[2][:], AF.Tanh,
                                 bias=biasg_sb[:, 2:3])
            nc.scalar.activation(o_s[:], g_ps[3][:], AF.Sigmoid,
                                 bias=biasg_sb[:, 3:4])
            t1 = lstmp.tile([HBL, B], f32, tag="lstm", name="t1")
            nc.vector.tensor_mul(t1[:], f_s[:], cT_sb[:])
            t2 = lstmp.tile([HBL, B], f32, tag="lstm", name="t2")
            nc.vector.tensor_mul(t2[:], i_s[:], gg[:])
            cN = lstmp.tile([HBL, B], f32, tag="lstm", name="cN")
            nc.vector.tensor_add(cN[:], t1[:], t2[:])
            tcn = lstmp.tile([HBL, B], f32, tag="lstm", name="tcn")
            nc.scalar.activation(tcn[:], cN[:], AF.Tanh)
            hN = lstmp.tile([HBL, B], f32, tag="lstm", name="hN")
            nc.vector.tensor_mul(hN[:], o_s[:], tcn[:])
            hN16 = lstmp.tile([HBL, B], f16, tag="lstm", name="hN16")
            nc.vector.tensor_copy(hN16[:], hN[:])

            # natural-layout outputs for the h/c block
            cn_ps = psp.tile([B, HBL], f32, tag="ps", name="cn_ps")
            nc.tensor.transpose(cn_ps[:], cN[:], ident32[:])
            c_nat = lstmp.tile([B, HBL], f32, tag="lstm", name="c_nat")
            nc.scalar.copy(c_nat[:], cn_ps[:])
            nc.sync.dma_start(c_sh.ap()[:, :], c_nat[:])
            hn_ps = psp.tile([B, HBL], f32, tag="ps", name="hn_ps")
            nc.tensor.transpose(hn_ps[:], hN[:], ident32[:])
            h_nat = lstmp.tile([B, HBL], f32, tag="lstm", name="h_nat")
            nc.scalar.copy(h_nat[:], hn_ps[:])
            nc.sync.dma_start(h_sh.ap()[:, :], h_nat[:])

            # ======== AllGather h_new^T blocks -> full [H, B] fp16 ========
            cc2_in = dram.tile([HBL, B], f16)
            cc2_out = dram.tile([H, B], f16, addr_space="Shared")
            nc.sync.dma_start(cc2_in[:], hN16[:])
            nc.gpsimd.collective_compute(
                "AllGather", mybir.AluOpType.bypass,
                replica_groups=[list(range(NC))],
                ins=[cc2_in[:].opt()], outs=[cc2_out[:].opt()])
            hTnew_sb = setup.tile([128, KT_H, B], f16)
            nc.sync.dma_start(
                hTnew_sb[:], cc2_out.rearrange("(kt p) b -> p kt b", p=128))

            # ======== fc: logits shard [B, VS] ========
            cat_tiles = ([hTnew_sb[:, kt, :] for kt in range(KT_H)]
                         + [ctxT_sb[:, kt, :] for kt in range(KT_H)])
            fc_ps = []
            offs = []
            off = 0
            for nch, w in enumerate(FC_CH):
                fc_ps.append(psp.tile([B, 512], f32, tag="ps", name=f"fc_ps{nch}"))
                offs.append(off)
                off += w
            for kt in range(KT_2H):
                for nch, w in enumerate(FC_CH):
                    nc.tensor.matmul(fc_ps[nch][:, :w], cat_tiles[kt],
                                     fcw_sb[kt][:, bass.ds(offs[nch], w)],
                                     start=(kt == 0), stop=False)
            with tc.tile_pool(name="fcbp", bufs=2) as fcbp:
                for nch, w in enumerate(FC_CH):
                    fcb_t = fcbp.tile([1, 512], f16, name="fcb_t")
                    nc.sync.dma_start(fcb_t[:, :w],
                                      fcb.ap()[:, bass.ds(offs[nch], w)])
                    nc.tensor.matmul(fc_ps[nch][:, :w], ones16[:, :B],
                                     fcb_t[:, :w],
                                     start=False, stop=True)
            for nch, w in enumerate(FC_CH):
                lg = lgp.tile([B, 512], f32, name="lg")
                nc.scalar.copy(lg[:, :w], fc_ps[nch][:, :w])
                nc.sync.dma_start(logits_sh.ap()[:, bass.ds(offs[nch], w)],
                                  lg[:, :w])

    nc.compile()
    return nc


def _prep_inputs(x, hidden, cell, encoder_outputs, emb_table, attn_W, attn_b,
                 v, W_ih, W_hh, b_ih, b_hh, fc_W, fc_b):
    f16, f32 = np.float16, np.float32
    x = np.asarray(x).astype(np.int32).reshape(B, 1)
    hidden = np.asarray(hidden, dtype=f32)
    cell = np.asarray(cell, dtype=f32)
    enc = np.asarray(encoder_outputs, dtype=f32)
    emb_table = np.ascontiguousarray(np.asarray(emb_table, dtype=f32))
    attn_W = np.asarray(attn_W, dtype=f32)
    attn_b = np.asarray(attn_b, dtype=f32)
    v = np.asarray(v, dtype=f32)
    W_ih = np.asarray(W_ih, dtype=f32)
    W_hh = np.asarray(W_hh, dtype=f32)
    b_ih = np.asarray(b_ih, dtype=f32)
    b_hh = np.asarray(b_hh, dtype=f32)
    fc_W = np.asarray(fc_W, dtype=f32)
    fc_b = np.asarray(fc_b, dtype=f32)

    hT = hidden[0].T                                   # [H, B]
    cT = cell[0].T                                     # [H, B]
    w1t_v = np.ascontiguousarray(attn_W[:, :H].T, dtype=f16)
    w2t_v = np.ascontiguousarray(attn_W[:, H:].T, dtype=f16)
    Wfull = np.concatenate([W_ih, W_hh], axis=1)       # [4H, E+2H]
    bsum = b_ih + b_hh

    common = {
        "x_idx": x,
        "emb_tab": emb_table,
        "hT_full": np.ascontiguousarray(hT, dtype=f16),
        "w1t": w1t_v,
        "w2t": w2t_v,
        "attnb": attn_b[None, :].astype(f16),
        "v_row": v[None, :].astype(f16),
    }
    in_maps = []
    for c in range(NC):
        bs = slice(c * BL, (c + 1) * BL)
        hs = slice(c * HBL, (c + 1) * HBL)
        vs = slice(c * VS, (c + 1) * VS)
        rows = np.concatenate([np.arange(g * H + c * HBL, g * H + (c + 1) * HBL)
                               for g in range(4)])
        in_maps.append(dict(
            common,
            hT_loc=np.ascontiguousarray(hT[:, bs], dtype=f16),
            cT_blk=np.ascontiguousarray(cT[hs, :], dtype=f32),
            enc_nat=np.ascontiguousarray(enc[bs], dtype=f16).reshape(BL * S, H),
            encT=np.ascontiguousarray(
                enc[bs].transpose(0, 2, 1), dtype=f16).reshape(BL * H, S),
            wcombT=np.ascontiguousarray(Wfull[rows, :].T, dtype=f16),
            bias_g=np.ascontiguousarray(bsum[rows].reshape(4, HBL).T,
                                        dtype=f32),
            fcwT=np.ascontiguousarray(fc_W[vs, :].T, dtype=f16),
            fcb=fc_b[vs][None, :].astype(f16),
        ))
    return in_maps


def kernel(**inputs):
    if "nc" not in _CACHE:
        _CACHE["nc"] = _build()
    nc = _CACHE["nc"]
    in_maps = _prep_inputs(**inputs)
    res = run_bass_kernel_spmd(nc, in_maps, core_ids=list(range(NC))).results
    logits = np.concatenate([res[c]["logits_sh"] for c in range(NC)], axis=1)
    attnw = np.concatenate([res[c]["attnw_sh"] for c in range(NC)], axis=0)
    h_new = np.concatenate([res[c]["h_sh"] for c in range(NC)], axis=1)[None]
    c_new = np.concatenate([res[c]["c_sh"] for c in range(NC)], axis=1)[None]
    return (logits.astype(np.float32), h_new.astype(np.float32),
            c_new.astype(np.float32), attnw.astype(np.float32))


# revision 19
# speedup vs baseline: 1.0662x; 1.0662x over previous
"""Trainium2 Bass kernel for nn_Decoder (single-step attention LSTM decoder).

B=128, S=128, H=1024, E=512, V=32000, 8 NeuronCores.

Sharding:
  - attention: data-parallel over batch (16 rows per core)
  - LSTM: sharded over hidden units (128 h-rows per core), AllGather of context
    before and of h_new after
  - fc/logits + embedding table: vocab-parallel (4000 rows per core)

All matmuls run in fp16 (full PE rate, ~2^-11 rounding); accumulation is fp32
in PSUM. The attention score path (tanh -> v-dot -> softmax) is kept in fp32.

Host side only reshapes/transposes/casts/shards; all model math (embedding
gather included) runs on device.
"""
import numpy as np

import concourse.bass as bass
import concourse.mybir as mybir
import concourse.tile as tile
from concourse import bacc
from concourse.bass_utils import run_bass_kernel_spmd
from concourse.masks import make_identity

B, S, H, E, V = 128, 128, 1024, 512, 32000
NC = 8
BL = B // NC          # 16 local batch rows
HBL = H // NC         # 128 hidden rows per core (LSTM shard)
VS = V // NC          # 4000 vocab rows per core
KT_H = H // 128       # 8 k-tiles over H
KT_E = E // 128       # 4 k-tiles over E
KT_X = (E + 2 * H) // 128   # 20 k-tiles over [emb, context, h_last]
KT_2H = (2 * H) // 128      # 16 k-tiles over [h_new, context]
FC_CH = [512] * 7 + [VS - 512 * 7]   # psum-bank chunks of the 4000-wide shard

_CACHE = {}


def _build():
    nc = bacc.Bacc("TRN2", target_bir_lowering=False, debug=False,
                   num_devices=NC)
    dt = mybir.dt
    f32, f16, i32 = dt.float32, dt.float16, dt.int32
    AF = mybir.ActivationFunctionType
    OP = mybir.AluOpType

    # ---- I/O ----
    x_idx = nc.dram_tensor("x_idx", [B, 1], i32, kind="ExternalInput")
    emb_tab = nc.dram_tensor("emb_tab", [V, E], f32, kind="ExternalInput")
    hT_loc = nc.dram_tensor("hT_loc", [H, BL], f16, kind="ExternalInput")
    hT_full = nc.dram_tensor("hT_full", [H, B], f16, kind="ExternalInput")
    cT_blk = nc.dram_tensor("cT_blk", [HBL, B], f32, kind="ExternalInput")
    enc_nat = nc.dram_tensor("enc_nat", [BL * S, H], f16, kind="ExternalInput")
    encT = nc.dram_tensor("encT", [BL * H, S], f16, kind="ExternalInput")
    w1t = nc.dram_tensor("w1t", [H, H], f16, kind="ExternalInput")
    w2t = nc.dram_tensor("w2t", [H, H], f16, kind="ExternalInput")
    attnb = nc.dram_tensor("attnb", [1, H], f16, kind="ExternalInput")
    v_row = nc.dram_tensor("v_row", [1, H], f16, kind="ExternalInput")
    wcombT = nc.dram_tensor("wcombT", [E + 2 * H, 4 * HBL], f16,
                            kind="ExternalInput")
    bias_g = nc.dram_tensor("bias_g", [HBL, 4], f32, kind="ExternalInput")
    fcwT = nc.dram_tensor("fcwT", [2 * H, VS], f16, kind="ExternalInput")
    fcb = nc.dram_tensor("fcb", [1, VS], f16, kind="ExternalInput")

    logits_sh = nc.dram_tensor("logits_sh", [B, VS], f32, kind="ExternalOutput")
    attnw_sh = nc.dram_tensor("attnw_sh", [BL, S], f32, kind="ExternalOutput")
    h_sh = nc.dram_tensor("h_sh", [B, HBL], f32, kind="ExternalOutput")
    c_sh = nc.dram_tensor("c_sh", [B, HBL], f32, kind="ExternalOutput")

    with tile.TileContext(nc) as tc:
        with (
            tc.tile_pool(name="setup", bufs=1) as setup,
            tc.tile_pool(name="fcw", bufs=1) as fcwp,
            tc.tile_pool(name="w2p", bufs=1) as w2p,

            tc.tile_pool(name="encp", bufs=3) as encp,
            tc.tile_pool(name="natp", bufs=3) as natp,
            tc.tile_pool(name="energyp", bufs=2) as energyp,
            tc.tile_pool(name="p1bp", bufs=2) as p1bp,
            tc.tile_pool(name="wcp", bufs=4) as wcp,
            tc.tile_pool(name="lgp", bufs=2) as lgp,
            tc.tile_pool(name="smallp", bufs=1) as smallp,
            tc.tile_pool(name="lstmp", bufs=8) as lstmp,
            tc.tile_pool(name="psp", bufs=8, space="PSUM") as psp,
            tc.tile_pool(name="dram", bufs=1, space="DRAM") as dram,
        ):
            # ======== setup ========
            # fc weights: allocated up front, DMAd lazily (scalar queue)
            fcw_sb = [fcwp.tile([128, VS], f16, name=f"fcw{kt}")
                      for kt in range(KT_2H)]

            ident32 = setup.tile([128, 128], f32)
            make_identity(nc, ident32[:])
            ones16 = setup.tile([1, 128], f16)
            nc.gpsimd.memset(ones16[:], 1.0)

            earlyp_cm = tc.tile_pool(name="earlyp", bufs=1)
            earlyp = earlyp_cm.__enter__()
            v_sb = earlyp.tile([1, H], f16)
            nc.sync.dma_start(v_sb[:], v_row.ap()[:, :])
            attnb_sb = earlyp.tile([1, H], f16)
            nc.sync.dma_start(attnb_sb[:], attnb.ap()[:, :])
            hTl_sb = setup.tile([128, KT_H, BL], f16)
            nc.sync.dma_start(
                hTl_sb[:], hT_loc.ap().rearrange("(kt p) b -> p kt b", p=128))
            hTf_sb = setup.tile([128, KT_H, B], f16)
            nc.sync.dma_start(
                hTf_sb[:], hT_full.ap().rearrange("(kt p) b -> p kt b", p=128))
            cT_sb = setup.tile([HBL, B], f32)
            nc.sync.dma_start(cT_sb[:], cT_blk.ap()[:, :])
            biasg_sb = setup.tile([HBL, 4], f32)
            nc.sync.dma_start(biasg_sb[:], bias_g.ap()[:, :])

            # embedding gather + transpose to [E, B] fp16 k-tiles
            idx_sb = setup.tile([B, 1], i32)
            nc.sync.dma_start(idx_sb[:], x_idx.ap()[:, :])
            embT_sb = setup.tile([128, KT_E, B], f16)
            with tc.tile_pool(name="embp", bufs=1) as embp:
                emb_bt = embp.tile([B, E], f32)
                nc.gpsimd.indirect_dma_start(
                    out=emb_bt[:], out_offset=None, in_=emb_tab.ap()[:, :],
                    in_offset=bass.IndirectOffsetOnAxis(ap=idx_sb[:, :1],
                                                        axis=0))
                for et in range(KT_E):
                    p = psp.tile([128, B], f32, tag="ps", name="embT_ps")
                    nc.tensor.transpose(p[:], emb_bt[:, bass.ts(et, 128)],
                                        ident32[:])
                    nc.scalar.copy(embT_sb[:, et, :], p[:])

            # W2^T resident: [128, kt, H]
            w2_sb = w2p.tile([128, KT_H, H], f16)
            for kt in range(KT_H):
                nc.sync.dma_start(w2_sb[:, kt, :],
                                  w2t.ap()[kt * 128:(kt + 1) * 128, :])

            # v broadcast to all partitions: ones^T @ v_row
            vb_sb = setup.tile([128, H], f16)
            for ch in range(2):
                vb_ps = psp.tile([128, 512], f32, tag="ps", name="vb_ps")
                nc.tensor.matmul(vb_ps[:], ones16[:, :128],
                                 v_sb[:, bass.ts(ch, 512)], start=True, stop=True)
                nc.vector.tensor_copy(vb_sb[:, bass.ts(ch, 512)], vb_ps[:])

            # ======== part1b: (h_last @ W1^T + attn_b) for local rows ========
            p1_rows = setup.tile([BL, H], f16)
            p1_chs = [psp.tile([BL, 512], f32, tag="ps", name=f"p1_ps{ch}")
                      for ch in range(2)]
            with tc.tile_pool(name="w1p", bufs=2) as w1p:
                for kt in range(KT_H):
                    w1_t = w1p.tile([128, H], f16, name="w1t_t")
                    nc.sync.dma_start(w1_t[:],
                                      w1t.ap()[kt * 128:(kt + 1) * 128, :])
                    for ch in range(2):
                        nc.tensor.matmul(p1_chs[ch][:],
                                         hTl_sb[:, kt, :],
                                         w1_t[:, bass.ts(ch, 512)],
                                         start=(kt == 0), stop=False)
            for ch in range(2):
                nc.tensor.matmul(p1_chs[ch][:], ones16[:, :BL],
                                 attnb_sb[:, bass.ts(ch, 512)],
                                 start=False, stop=True)
                nc.scalar.copy(p1_rows[:, bass.ts(ch, 512)], p1_chs[ch][:])
            earlyp_cm.__exit__(None, None, None)

            # ======== attention: energy / score per local row ========
            score_sb = setup.tile([128, BL], f32)
            for b in range(BL):
                encT_b = encp.tile([128, KT_H, S], f16, name="encT_b")
                nc.sync.dma_start(
                    encT_b[:],
                    encT.ap()[b * H:(b + 1) * H, :]
                    .rearrange("(kt p) s -> p kt s", p=128))
                nc.scalar.dma_start(fcw_sb[b][:],
                                    fcwT.ap()[b * 128:(b + 1) * 128, :])
                # this row's part1 bias, moved onto partition 0
                p1b = p1bp.tile([1, H], f16, name="p1b")
                nc.sync.dma_start(p1b[:], p1_rows[b:b + 1, :])

                energy = energyp.tile([S, H], f16, name="energy")
                for ch in range(2):
                    pre_ps = psp.tile([S, 512], f32, tag="ps", name="pre_ps")
                    for kt in range(KT_H):
                        nc.tensor.matmul(
                            pre_ps[:], encT_b[:, kt, :],
                            w2_sb[:, kt, bass.ds(ch * 512, 512)],
                            start=(kt == 0), stop=False)
                    nc.tensor.matmul(
                        pre_ps[:], ones16[:, :S],
                        p1b[:, bass.ds(ch * 512, 512)],
                        start=False, stop=True)
                    nc.scalar.activation(energy[:, bass.ts(ch, 512)],
                                         pre_ps[:], AF.Tanh)
                nc.vector.tensor_mul(energy[:], energy[:], vb_sb[:])
                nc.vector.reduce_sum(out=score_sb[:, b:b + 1], in_=energy[:],
                                     axis=mybir.AxisListType.X)

            # ======== softmax over S (rows are local b) ========
            sc_ps = psp.tile([BL, S], f32, tag="ps", name="sc_ps")
            nc.tensor.transpose(sc_ps[:], score_sb[:, :BL], ident32[:])
            sc16 = smallp.tile([BL, S], f32)
            nc.scalar.copy(sc16[:], sc_ps[:])
            mx = smallp.tile([BL, 1], f32)
            nc.vector.reduce_max(out=mx[:], in_=sc16[:],
                                 axis=mybir.AxisListType.X)
            mxn = smallp.tile([BL, 1], f32)
            nc.scalar.mul(mxn[:], mx[:], -1.0)
            esc = smallp.tile([BL, S], f32)
            nc.scalar.activation(esc[:], sc16[:], AF.Exp, bias=mxn[:, :1])
            sm = smallp.tile([BL, 1], f32)
            nc.vector.reduce_sum(out=sm[:], in_=esc[:],
                                 axis=mybir.AxisListType.X)
            rs = smallp.tile([BL, 1], f32)
            nc.vector.reciprocal(rs[:], sm[:])
            w16 = smallp.tile([BL, S], f32)
            nc.vector.tensor_scalar_mul(w16[:], esc[:], rs[:, :1])
            nc.sync.dma_start(attnw_sh.ap()[:, :], w16[:])

            # ======== context = attn_weights @ enc ========
            wt_ps = psp.tile([S, BL], f32, tag="ps", name="wt_ps")
            nc.tensor.transpose(wt_ps[:], w16[:], ident32[:BL, :BL])
            wT = smallp.tile([S, BL], f16)
            nc.scalar.copy(wT[:], wt_ps[:])
            cc1_in = dram.tile([BL, H], f32)
            for b in range(BL):
                enc_b = natp.tile([S, H], f16, name="enc_b")
                nc.sync.dma_start(enc_b[:], enc_nat.ap()[b * S:(b + 1) * S, :])
                for ch in range(2):
                    ctx_ps = psp.tile([1, 512], f32, tag="ps", name="ctx_ps")
                    nc.tensor.matmul(ctx_ps[:],
                                     wT[:, b:b + 1],
                                     enc_b[:, bass.ts(ch, 512)],
                                     start=True, stop=True)
                    ctmp = p1bp.tile([1, 512], f32, tag="ctmp", name="ctmp")
                    nc.scalar.copy(ctmp[:], ctx_ps[:])
                    nc.sync.dma_start(
                        cc1_in[b:b + 1, bass.ds(ch * 512, 512)], ctmp[:])

            # ======== AllGather context -> full [B, H] ========
            cc1_out = dram.tile([B, H], f32, addr_space="Shared")
            nc.gpsimd.collective_compute(
                "AllGather", mybir.AluOpType.bypass,
                replica_groups=[list(range(NC))],
                ins=[cc1_in[:].opt()], outs=[cc1_out[:].opt()])
            ctx_full = setup.tile([B, H], f32)
            nc.sync.dma_start(ctx_full[:], cc1_out[:])
            ctxT_sb = setup.tile([128, KT_H, B], f16)
            for ht in range(KT_H):
                p = psp.tile([128, B], f32, tag="ps", name="ctxT_ps")
                nc.tensor.transpose(p[:], ctx_full[:, bass.ts(ht, 128)],
                                    ident32[:])
                nc.scalar.copy(ctxT_sb[:, ht, :], p[:])

            # ======== LSTM (gates sharded over h-block) ========
            rhs_tiles = ([embT_sb[:, et, :] for et in range(KT_E)]
                         + [ctxT_sb[:, ht, :] for ht in range(KT_H)]
                         + [hTf_sb[:, ht, :] for ht in range(KT_H)])
            g_ps = [psp.tile([HBL, B], f32, tag="ps", name=f"g_ps{g}")
                    for g in range(4)]
            for kt in range(KT_X):
                wc_t = wcp.tile([128, 4 * HBL], f16, name="wc_t")
                nc.sync.dma_start(wc_t[:],
                                  wcombT.ap()[kt * 128:(kt + 1) * 128, :])
                for g in range(4):
                    nc.tensor.matmul(g_ps[g][:], wc_t[:, bass.ts(g, HBL)],
                                     rhs_tiles[kt],
                                     start=(kt == 0), stop=(kt == KT_X - 1))
            i_s = lstmp.tile([HBL, B], f32, tag="lstm", name="i_s")
            f_s = lstmp.tile([HBL, B], f32, tag="lstm", name="f_s")
            gg = lstmp.tile([HBL, B], f32, tag="lstm", name="gg")
            o_s = lstmp.tile([HBL, B], f32, tag="lstm", name="o_s")
            nc.scalar.activation(i_s[:], g_ps[0][:], AF.Sigmoid,
                                 bias=biasg_sb[:, 0:1])
            nc.scalar.activation(f_s[:], g_ps[1][:], AF.Sigmoid,
                                 bias=biasg_sb[:, 1:2])
            nc.scalar.activation(gg[:], g_ps[2][:], AF.Tanh,
                                 bias=biasg_sb[:, 2:3])
            nc.scalar.activation(o_s[:], g_ps[3][:], AF.Sigmoid,
                                 bias=biasg_sb[:, 3:4])
            t1 = lstmp.tile([HBL, B], f32, tag="lstm", name="t1")
            nc.vector.tensor_mul(t1[:], f_s[:], cT_sb[:])
            t2 = lstmp.tile([HBL, B], f32, tag="lstm", name="t2")
            nc.vector.tensor_mul(t2[:], i_s[:], gg[:])
            cN = lstmp.tile([HBL, B], f32, tag="lstm", name="cN")
            nc.vector.tensor_add(cN[:], t1[:], t2[:])
            tcn = lstmp.tile([HBL, B], f32, tag="lstm", name="tcn")
            nc.scalar.activation(tcn[:], cN[:], AF.Tanh)
            hN = lstmp.tile([HBL, B], f32, tag="lstm", name="hN")
            nc.vector.tensor_mul(hN[:], o_s[:], tcn[:])
            hN16 = lstmp.tile([HBL, B], f16, tag="lstm", name="hN16")
            nc.vector.tensor_copy(hN16[:], hN[:])

            # natural-layout outputs for the h/c block
            cn_ps = psp.tile([B, HBL], f32, tag="ps", name="cn_ps")
            nc.tensor.transpose(cn_ps[:], cN[:], ident32[:])
            c_nat = lstmp.tile([B, HBL], f32, tag="lstm", name="c_nat")
            nc.scalar.copy(c_nat[:], cn_ps[:])
            nc.sync.dma_start(c_sh.ap()[:, :], c_nat[:])
            hn_ps = psp.tile([B, HBL], f32, tag="ps", name="hn_ps")
            nc.tensor.transpose(hn_ps[:], hN[:], ident32[:])
            h_nat = lstmp.tile([B, HBL], f32, tag="lstm", name="h_nat")
            nc.scalar.copy(h_nat[:], hn_ps[:])
            nc.sync.dma_start(h_sh.ap()[:, :], h_nat[:])

            # ======== AllGather h_new^T blocks -> full [H, B] fp16 ========
            cc2_in = dram.tile([HBL, B], f16)
            cc2_out = dram.tile([H, B], f16, addr_space="Shared")
            nc.sync.dma_start(cc2_in[:], hN16[:])
            nc.gpsimd.collective_compute(
                "AllGather", mybir.AluOpType.bypass,
                replica_groups=[list(range(NC))],
                ins=[cc2_in[:].opt()], outs=[cc2_out[:].opt()])
            hTnew_sb = setup.tile([128, KT_H, B], f16)
            nc.sync.dma_start(
                hTnew_sb[:], cc2_out.rearrange("(kt p) b -> p kt b", p=128))

            # ======== fc: logits shard [B, VS] ========
            cat_tiles = ([hTnew_sb[:, kt, :] for kt in range(KT_H)]
                         + [ctxT_sb[:, kt, :] for kt in range(KT_H)])
            fc_ps = []
            offs = []
            off = 0
            for nch, w in enumerate(FC_CH):
                fc_ps.append(psp.tile([B, 512], f32, tag="ps", name=f"fc_ps{nch}"))
                offs.append(off)
                off += w
            for kt in range(KT_2H):
                for nch, w in enumerate(FC_CH):
                    nc.tensor.matmul(fc_ps[nch][:, :w], cat_tiles[kt],
                                     fcw_sb[kt][:, bass.ds(offs[nch], w)],
                                     start=(kt == 0), stop=False)
            with tc.tile_pool(name="fcbp", bufs=2) as fcbp:
                for nch, w in enumerate(FC_CH):
                    fcb_t = fcbp.tile([1, 512], f16, name="fcb_t")
                    nc.sync.dma_start(fcb_t[:, :w],
                                      fcb.ap()[:, bass.ds(offs[nch], w)])
                    nc.tensor.matmul(fc_ps[nch][:, :w], ones16[:, :B],
                                     fcb_t[:, :w],
                                     start=False, stop=True)
            for nch, w in enumerate(FC_CH):
                lg = lgp.tile([B, 512], f32, name="lg")
                nc.scalar.copy(lg[:, :w], fc_ps[nch][:, :w])
                nc.sync.dma_start(logits_sh.ap()[:, bass.ds(offs[nch], w)],
                                  lg[:, :w])

    nc.compile()
    return nc


def _prep_inputs(x, hidden, cell, encoder_outputs, emb_table, attn_W, attn_b,
                 v, W_ih, W_hh, b_ih, b_hh, fc_W, fc_b):
    f16, f32 = np.float16, np.float32
    x = np.asarray(x).astype(np.int32).reshape(B, 1)
    hidden = np.asarray(hidden, dtype=f32)
    cell = np.asarray(cell, dtype=f32)
    enc = np.asarray(encoder_outputs, dtype=f32)
    emb_table = np.ascontiguousarray(np.asarray(emb_table, dtype=f32))
    attn_W = np.asarray(attn_W, dtype=f32)
    attn_b = np.asarray(attn_b, dtype=f32)
    v = np.asarray(v, dtype=f32)
    W_ih = np.asarray(W_ih, dtype=f32)
    W_hh = np.asarray(W_hh, dtype=f32)
    b_ih = np.asarray(b_ih, dtype=f32)
    b_hh = np.asarray(b_hh, dtype=f32)
    fc_W = np.asarray(fc_W, dtype=f32)
    fc_b = np.asarray(fc_b, dtype=f32)

    hT = hidden[0].T                                   # [H, B]
    cT = cell[0].T                                     # [H, B]
    w1t_v = np.ascontiguousarray(attn_W[:, :H].T, dtype=f16)
    w2t_v = np.ascontiguousarray(attn_W[:, H:].T, dtype=f16)
    Wfull = np.concatenate([W_ih, W_hh], axis=1)       # [4H, E+2H]
    bsum = b_ih + b_hh

    common = {
        "x_idx": x,
        "emb_tab": emb_table,
        "hT_full": np.ascontiguousarray(hT, dtype=f16),
        "w1t": w1t_v,
        "w2t": w2t_v,
        "attnb": attn_b[None, :].astype(f16),
        "v_row": v[None, :].astype(f16),
    }
    in_maps = []
    for c in range(NC):
        bs = slice(c * BL, (c + 1) * BL)
        hs = slice(c * HBL, (c + 1) * HBL)
        vs = slice(c * VS, (c + 1) * VS)
        rows = np.concatenate([np.arange(g * H + c * HBL, g * H + (c + 1) * HBL)
                               for g in range(4)])
        in_maps.append(dict(
            common,
            hT_loc=np.ascontiguousarray(hT[:, bs], dtype=f16),
            cT_blk=np.ascontiguousarray(cT[hs, :], dtype=f32),
            enc_nat=np.ascontiguousarray(enc[bs], dtype=f16).reshape(BL * S, H),
            encT=np.ascontiguousarray(
                enc[bs].transpose(0, 2, 1), dtype=f16).reshape(BL * H, S),
            wcombT=np.ascontiguousarray(Wfull[rows, :].T, dtype=f16),
            bias_g=np.ascontiguousarray(bsum[rows].reshape(4, HBL).T,
                                        dtype=f32),
            fcwT=np.ascontiguousarray(fc_W[vs, :].T, dtype=f16),
            fcb=fc_b[vs][None, :].astype(f16),
        ))
    return in_maps


def kernel(**inputs):
    if "nc" not in _CACHE:
        _CACHE["nc"] = _build()
    nc = _CACHE["nc"]
    in_maps = _prep_inputs(**inputs)
    res = run_bass_kernel_spmd(nc, in_maps, core_ids=list(range(NC))).results
    logits = np.concatenate([res[c]["logits_sh"] for c in range(NC)], axis=1)
    attnw = np.concatenate([res[c]["attnw_sh"] for c in range(NC)], axis=0)
    h_new = np.concatenate([res[c]["h_sh"] for c in range(NC)], axis=1)[None]
    c_new = np.concatenate([res[c]["c_sh"] for c in range(NC)], axis=1)[None]
    return (logits.astype(np.float32), h_new.astype(np.float32),
            c_new.astype(np.float32), attnw.astype(np.float32))
